# revision 22
# baseline (speedup 1.0000x reference)
"""MoE gate (LLaDA2) routing kernel for 8 Trainium2 NeuronCores.

Token-parallel over 8 cores (2048 tokens/core, 16 tiles of 128). Router GEMM
runs as an fp16 main pass (xh @ w16*2^19) plus a DoubleRow fp8 correction
(x8@wl8*2^19 + xl8*2^12 @ w8*2^7), all accumulating into ONE PSUM bank per
tile at 2^19 scale, so the sigmoid reads PSUM directly with scale=2^-19 and
no combine/copy ops are needed. w8 (= fp8(w*2^7)) is derived on-chip once.
PE schedule: warm-up junk matmuls (HAM clock-gate release) -> 8 main passes
run ahead of their corrections (8 open PSUM banks) -> D/m interleave ->
correction drain. Grouped top-8 routing epilogue: all 256-wide work on DVE
(GPSIMD is ~8ns/elem/partition -- only <=64-elem ops go to Pool), ACT does
the sigmoid, Pool does the K x K reorder trees + normalize_recip.
"""
import sys
for p in ("/opt/trn_rl_repo", "/root/.axon_site/_ro/trn_rl_repo"):
    if p not in sys.path:
        sys.path.append(p)

import numpy as np

T, H, E = 16384, 4096, 256
NCORES = 8
TPC = T // NCORES          # tokens per core: 2048
NTILES = TPC // 128        # 16 row tiles
KCH = H // 128             # 32 contraction chunks
KQ = KCH // 4              # chunks per w16 quarter
G = 8                      # expert groups
GS = E // G                # 32 experts/group
K = 8                      # top-k
BIG = 2.0 ** 100
NEG = -1.0e4
SCALE = 2.0 ** -19         # undo the 2^19 logit scaling at sigmoid time
NWARM = 30                 # junk matmuls to release the PE HAM clock gate
AHEAD = 8                  # main passes run before the first correction pass

_cache = {}


def _build():
    import concourse.bacc as bacc
    import concourse.bass as bass
    import concourse.mybir as mybir
    from concourse import tile

    dt = mybir.dt
    Alu = mybir.AluOpType
    Act = mybir.ActivationFunctionType
    Ax = mybir.AxisListType
    DR = mybir.MatmulPerfMode.DoubleRow

    nc = bacc.Bacc("TRN2", target_bir_lowering=False, debug=False,
                   num_devices=NCORES)

    xh_d = nc.dram_tensor("xh", [NTILES, 128, KCH, 128], dt.float16, kind="ExternalInput")
    xl_d = nc.dram_tensor("xl", [NTILES, 128, KCH, 128], dt.float8e4, kind="ExternalInput")
    x8_d = nc.dram_tensor("x8", [NTILES, 128, KCH, 128], dt.float8e4, kind="ExternalInput")
    w16_d = nc.dram_tensor("w16", [128, KCH, E], dt.float16, kind="ExternalInput")
    wl_d = nc.dram_tensor("wl", [128, KCH, E], dt.float8e4, kind="ExternalInput")
    btab_d = nc.dram_tensor("btab", [128, E], dt.float32, kind="ExternalInput")
    w_out = nc.dram_tensor("w_out", [TPC, K], dt.float32, kind="ExternalOutput")
    i_out = nc.dram_tensor("i_out", [TPC, K], dt.uint32, kind="ExternalOutput")

    def bc_mid(ap8, n=8):
        # [128, m] -> [128, n(bcast), m]
        return bass.AP(ap8.tensor, ap8.offset, [list(ap8.ap[0]), [0, n], list(ap8.ap[1])])

    with tile.TileContext(nc) as tc:
        with (
            tc.tile_pool(name="wpool", bufs=1) as wpool,
            tc.tile_pool(name="xpool", bufs=6) as xpool,
            tc.tile_pool(name="lpool", bufs=12) as lpool,
            tc.tile_pool(name="qpool", bufs=9) as qpool,
            tc.tile_pool(name="ppool", bufs=8, space="PSUM") as ppool,
            tc.tile_pool(name="spool", bufs=5) as spool,
            tc.tile_pool(name="tpool", bufs=5) as tpool,
            tc.tile_pool(name="opool", bufs=1) as opool,
            tc.tile_pool(name="dpool", bufs=16) as dpool,
        ):
            # ---------------- input DMAs, in delivery-priority order -------
            w16q = [wpool.tile([128, KQ * E], dt.float16, tag=f"w16q{q}", name=f"w16q{q}") for q in range(4)]
            w16_flat = w16_d[:].rearrange("p k e -> p (k e)")
            xts, xlts = [None] * NTILES, [None] * NTILES
            x8ts = [None] * NTILES

            def dma_x(i):
                xt = xpool.tile([128, KCH * 128], dt.float16, tag="x")
                nc.sync.dma_start(xt[:], xh_d[i].rearrange("p k t -> p (k t)"))
                xts[i] = xt

            def dma_xl(i):
                lt = lpool.tile([128, KCH * 128], dt.float8e4, tag="xl")
                nc.sync.dma_start(lt[:], xl_d[i].rearrange("p k t -> p (k t)"))
                xlts[i] = lt
                qt = qpool.tile([128, KCH * 128], dt.float8e4, tag="x8")
                nc.sync.dma_start(qt[:], x8_d[i].rearrange("p k t -> p (k t)"))
                x8ts[i] = qt

            nc.sync.dma_start(w16q[0][:], w16_flat[:, 0:KQ * E])
            dma_x(0)
            nc.sync.dma_start(w16q[1][:], w16_flat[:, KQ * E:2 * KQ * E])
            nc.sync.dma_start(w16q[2][:], w16_flat[:, 2 * KQ * E:3 * KQ * E])
            nc.sync.dma_start(w16q[3][:], w16_flat[:, 3 * KQ * E:4 * KQ * E])
            wl8 = wpool.tile([128, KCH * E], dt.float8e4, tag="wl8")
            wl_flat = wl_d[:].rearrange("p k e -> p (k e)")
            nc.sync.dma_start(wl8[:, 0:KCH * E // 2], wl_flat[:, 0:KCH * E // 2])
            nc.sync.dma_start(wl8[:, KCH * E // 2:], wl_flat[:, KCH * E // 2:])
            dma_xl(0)
            btab = wpool.tile([128, E], dt.float32, tag="btab")
            nc.sync.dma_start(btab[:], btab_d[:])
            for i in range(1, NTILES):
                dma_x(i)
                dma_xl(i)

            # ---------------- PE warm-up (no data deps) --------------------
            junk = wpool.tile([128, 384], dt.float16, tag="junk")
            nc.vector.memset(junk[:], 0.0)
            jp = ppool.tile([128, E], dt.float32, tag="ps")
            for _ in range(NWARM):
                nc.tensor.matmul(jp[:], lhsT=junk[:, 0:128], rhs=junk[:, 128:384],
                                 start=True, stop=True)

            # ---------------- on-chip fp8 derivations (DVE) ----------------
            # w8 = fp8(w * 2^7) = fp8(w16 * 2^-12), one-time
            w8d = wpool.tile([128, KCH * E], dt.float8e4, tag="w8d")
            for q in range(4):
                nc.vector.tensor_scalar(w8d[:, q * KQ * E:(q + 1) * KQ * E],
                                        w16q[q][:], 2.0 ** -12, None, op0=Alu.mult)

            psums = [None] * NTILES
            w8s = [None] * NTILES
            out_w = opool.tile([128, NTILES * K], dt.float32, tag="ow")
            dtile = opool.tile([128, NTILES], dt.float32, tag="dens")
            rtile = opool.tile([128, NTILES], dt.float32, tag="recs")
            out_i = opool.tile([128, NTILES * K], dt.uint32, tag="oi")

            def main_half(i, h):
                if h == 0:
                    psums[i] = ppool.tile([128, E], dt.float32, tag="ps", name="ps")
                ps = psums[i]
                x = xts[i]
                for k in range(h * KCH // 2, (h + 1) * KCH // 2):
                    nc.tensor.matmul(ps[:],
                                     lhsT=x[:, k * 128:(k + 1) * 128],
                                     rhs=w16q[k // KQ][:, (k % KQ) * E:(k % KQ + 1) * E],
                                     start=(k == 0), stop=False)

            def main_pass(i):
                main_half(i, 0)
                main_half(i, 1)

            def corr_pass(i):
                ps = psums[i]
                u3 = x8ts[i][:].rearrange("p (k t) -> p k t", k=KCH)
                l3 = xlts[i][:].rearrange("p (k t) -> p k t", k=KCH)
                va = wl8[:].rearrange("p (k e) -> p k e", k=KCH)
                vb = w8d[:].rearrange("p (k e) -> p k e", k=KCH)
                for j in range(KCH // 2):
                    nc.tensor.matmul(ps[:], lhsT=u3[:, 2 * j:2 * j + 2, :],
                                     rhs=va[:, 2 * j:2 * j + 2, :],
                                     start=False, stop=False, perf_mode=DR)
                for j in range(KCH // 2):
                    nc.tensor.matmul(ps[:], lhsT=l3[:, 2 * j:2 * j + 2, :],
                                     rhs=vb[:, 2 * j:2 * j + 2, :],
                                     start=False, stop=(j == KCH // 2 - 1), perf_mode=DR)

            def epilogue(i):
                ps = psums[i]
                scores = spool.tile([128, E], dt.float32, tag="scores")
                nc.scalar.activation(scores[:], ps[:], Act.Sigmoid, scale=SCALE)
                sr = spool.tile([128, E], dt.float32, tag="sr")
                nc.vector.tensor_tensor(sr[:], scores[:], btab[:], Alu.add)
                sr3 = sr[:].rearrange("p (g e) -> p g e", g=G)

                top1 = tpool.tile([128, G], dt.float32, tag="top1")
                nc.vector.tensor_reduce(top1[:], sr3, axis=Ax.X, op=Alu.max)
                mr2 = spool.tile([128, E], dt.float32, tag="mr2")
                nc.vector.match_replace(mr2[:], in_to_replace=top1[:], in_values=sr[:], imm_value=NEG)
                top2 = tpool.tile([128, G], dt.float32, tag="top2")
                nc.vector.tensor_reduce(top2[:], mr2[:].rearrange("p (g e) -> p g e", g=G), axis=Ax.X, op=Alu.max)
                gs_t = tpool.tile([128, G], dt.float32, tag="gs")
                nc.vector.tensor_tensor(gs_t[:], top1[:], top2[:], Alu.add)
                g8 = tpool.tile([128, 8], dt.float32, tag="g8")
                nc.vector.max(out=g8[:], in_=gs_t[:])
                # inv = 1e4 for groups strictly below the 4th-largest group score
                inv = tpool.tile([128, G], dt.float32, tag="inv")
                nc.vector.tensor_scalar(inv[:], gs_t[:], g8[:, 3:4], -NEG, op0=Alu.is_lt, op1=Alu.mult)
                nc.vector.tensor_tensor(sr3, sr3, inv[:].to_broadcast([128, G, GS]), Alu.subtract)

                vals8 = tpool.tile([128, K], dt.float32, tag="vals8")
                nc.vector.max(out=vals8[:], in_=sr[:])
                oi_c = out_i[:, i * K:(i + 1) * K]
                nc.vector.max_index(out=oi_c, in_max=vals8[:], in_values=sr[:])

                # sel = scores at the selected positions (threshold at the 8th value)
                sel = spool.tile([128, E], dt.float32, tag="sel")
                nc.vector.scalar_tensor_tensor(sel[:], in0=sr[:], scalar=vals8[:, 7:8],
                                               in1=scores[:], op0=Alu.is_ge, op1=Alu.mult)
                svals8 = tpool.tile([128, K], dt.float32, tag="svals8")
                nc.vector.max(out=svals8[:], in_=sel[:])
                sidx8 = tpool.tile([128, K], dt.uint32, tag="sidx8")
                nc.vector.max_index(out=sidx8[:], in_max=svals8[:], in_values=sel[:])

                # reorder svals8 (score order) into routing order: K x K match
                eq = tpool.tile([128, K * K], dt.float32, tag="eq")
                eq3 = eq[:].rearrange("p (k j) -> p k j", k=K)
                nc.vector.tensor_tensor(eq3, oi_c.to_broadcast([128, K, K]), bc_mid(sidx8[:]), Alu.is_equal)
                nc.gpsimd.tensor_tensor(eq3, eq3, bc_mid(svals8[:]), Alu.mult)
                w84 = tpool.tile([128, K * 4], dt.float32, tag="w84")
                w843 = w84[:].rearrange("p (k j) -> p k j", k=K)
                nc.gpsimd.tensor_tensor(w843, eq3[:, :, 0:4], eq3[:, :, 4:8], Alu.add)
                w82 = tpool.tile([128, K * 2], dt.float32, tag="w82")
                w823 = w82[:].rearrange("p (k j) -> p k j", k=K)
                nc.gpsimd.tensor_tensor(w823, w843[:, :, 0:2], w843[:, :, 2:4], Alu.add)
                w8 = dpool.tile([128, K], dt.float32, tag="w8")
                nc.gpsimd.tensor_tensor(w8[:].rearrange("p (k o) -> p k o", o=1),
                                        w823[:, :, 0:1], w823[:, :, 1:2], Alu.add)

                s4 = tpool.tile([128, 4], dt.float32, tag="s4")
                nc.gpsimd.tensor_tensor(s4[:], w8[:, 0:4], w8[:, 4:8], Alu.add)
                s2 = tpool.tile([128, 2], dt.float32, tag="s2")
                nc.gpsimd.tensor_tensor(s2[:], s4[:, 0:2], s4[:, 2:4], Alu.add)
                dc = dtile[:, i:i + 1]
                nc.gpsimd.tensor_tensor(dc, s2[:, 0:1], s2[:, 1:2], Alu.add)
                nc.gpsimd.tensor_scalar(dc, dc, 1.0 / 2.5, None, op0=Alu.mult)
                w8s[i] = w8

            # ---------------- schedule ------------------------------------
            # Per-tile [main, correction] pairs, corrections starting as early
            # as DMA allows so the (DVE-bound) epilogues get the widest
            # possible window. All normalize/scale finishes run at the very
            # end so the DVE stream never blocks mid-run on the Pool trees.
            for i in range(NTILES):
                main_pass(i)
                corr_pass(i)
                epilogue(i)
            for g in range(4):
                nc.vector.reciprocal(rtile[:, 4 * g:4 * g + 4], dtile[:, 4 * g:4 * g + 4])
                for t in range(4 * g, 4 * g + 4):
                    nc.scalar.activation(out_w[:, t * K:(t + 1) * K], w8s[t][:], Act.Copy,
                                         scale=rtile[:, t:t + 1])
                wo = w_out[:].rearrange("(i p) k -> p i k", p=128)[:, 4 * g:4 * g + 4, :]
                io = i_out[:].rearrange("(i p) k -> p i k", p=128)[:, 4 * g:4 * g + 4, :]
                nc.sync.dma_start(wo, out_w[:, 32 * g:32 * (g + 1)].rearrange("p (i k) -> p i k", i=4))
                nc.sync.dma_start(io, out_i[:, 32 * g:32 * (g + 1)].rearrange("p (i k) -> p i k", i=4))

    nc.compile()
    return nc


def _prep(hidden_states, weight, expert_bias):
    import ml_dtypes
    f8 = ml_dtypes.float8_e4m3
    x = np.ascontiguousarray(hidden_states, dtype=np.float32)
    w = np.ascontiguousarray(weight, dtype=np.float32)

    w16s = (w * 2.0 ** 19).astype(np.float16)            # shipped main weights
    wl = w - w16s.astype(np.float32) * 2.0 ** -19
    wl8 = (wl * 2.0 ** 19).astype(f8)                    # scaled residual of w
    w16_l = np.ascontiguousarray(w16s.reshape(E, KCH, 128).transpose(2, 1, 0))
    wl8_l = np.ascontiguousarray(wl8.reshape(E, KCH, 128).transpose(2, 1, 0))
    btab = np.ascontiguousarray(np.broadcast_to(expert_bias.astype(np.float32), (128, E)))

    in_maps = []
    for c in range(NCORES):
        xs = x[c * TPC:(c + 1) * TPC]
        xh16 = xs.astype(np.float16)
        xl = xs - xh16.astype(np.float32)
        xl8 = (xl * 2.0 ** 12).astype(f8)
        x8 = xs.astype(f8)
        xh_l = np.ascontiguousarray(xh16.reshape(NTILES, 128, KCH, 128).transpose(0, 3, 2, 1))
        xl8_l = np.ascontiguousarray(xl8.reshape(NTILES, 128, KCH, 128).transpose(0, 3, 2, 1))
        x8_l = np.ascontiguousarray(x8.reshape(NTILES, 128, KCH, 128).transpose(0, 3, 2, 1))
        in_maps.append({"xh": xh_l, "xl": xl8_l, "x8": x8_l, "w16": w16_l, "wl": wl8_l, "btab": btab})
    return in_maps


def kernel(hidden_states, weight, expert_bias, _trace=False):
    from concourse.bass_utils import run_bass_kernel_spmd

    if "nc" not in _cache:
        _cache["nc"] = _build()
    nc = _cache["nc"]
    in_maps = _prep(hidden_states, weight, expert_bias)
    res = run_bass_kernel_spmd(nc, in_maps, core_ids=list(range(NCORES)), trace=_trace)
    _cache["last_results"] = res
    w = np.concatenate([res.results[c]["w_out"] for c in range(NCORES)], axis=0)
    idx = np.concatenate([res.results[c]["i_out"] for c in range(NCORES)], axis=0)
    return w.astype(np.float32), idx.astype(np.int32)


# revision 23
# speedup vs baseline: 1.0153x; 1.0153x over previous
"""MoE gate (LLaDA2) routing kernel for 8 Trainium2 NeuronCores.

Token-parallel over 8 cores (2048 tokens/core, 16 tiles of 128). Router GEMM
runs as an fp16 main pass (xh @ w16*2^19) plus a DoubleRow fp8 correction
(x8@wl8*2^19 + xl8*2^12 @ w8*2^7), all accumulating into ONE PSUM bank per
tile at 2^19 scale, so the sigmoid reads PSUM directly with scale=2^-19 and
no combine/copy ops are needed. w8 (= fp8(w*2^7)) is derived on-chip once.
PE schedule: warm-up junk matmuls (HAM clock-gate release) -> 8 main passes
run ahead of their corrections (8 open PSUM banks) -> D/m interleave ->
correction drain. Grouped top-8 routing epilogue: all 256-wide work on DVE
(GPSIMD is ~8ns/elem/partition -- only <=64-elem ops go to Pool), ACT does
the sigmoid, Pool does the K x K reorder trees + normalize_recip.
"""
import sys
for p in ("/opt/trn_rl_repo", "/root/.axon_site/_ro/trn_rl_repo"):
    if p not in sys.path:
        sys.path.append(p)

import numpy as np

T, H, E = 16384, 4096, 256
NCORES = 8
TPC = T // NCORES          # tokens per core: 2048
NTILES = TPC // 128        # 16 row tiles
KCH = H // 128             # 32 contraction chunks
KQ = KCH // 4              # chunks per w16 quarter
G = 8                      # expert groups
GS = E // G                # 32 experts/group
K = 8                      # top-k
BIG = 2.0 ** 100
NEG = -1.0e4
SCALE = 2.0 ** -19         # undo the 2^19 logit scaling at sigmoid time
NWARM = 30                 # junk matmuls to release the PE HAM clock gate
AHEAD = 8                  # main passes run before the first correction pass

_cache = {}


def _build():
    import concourse.bacc as bacc
    import concourse.bass as bass
    import concourse.mybir as mybir
    from concourse import tile

    dt = mybir.dt
    Alu = mybir.AluOpType
    Act = mybir.ActivationFunctionType
    Ax = mybir.AxisListType
    DR = mybir.MatmulPerfMode.DoubleRow

    nc = bacc.Bacc("TRN2", target_bir_lowering=False, debug=False,
                   num_devices=NCORES)

    xh_d = nc.dram_tensor("xh", [NTILES, 128, KCH, 128], dt.float16, kind="ExternalInput")
    xl_d = nc.dram_tensor("xl", [NTILES, 128, KCH, 128], dt.float8e4, kind="ExternalInput")
    x8_d = nc.dram_tensor("x8", [NTILES, 128, KCH, 128], dt.float8e4, kind="ExternalInput")
    w16_d = nc.dram_tensor("w16", [128, KCH, E], dt.float16, kind="ExternalInput")
    wl_d = nc.dram_tensor("wl", [128, KCH, E], dt.float8e4, kind="ExternalInput")
    btab_d = nc.dram_tensor("btab", [128, E], dt.float32, kind="ExternalInput")
    w_out = nc.dram_tensor("w_out", [TPC, K], dt.float32, kind="ExternalOutput")
    i_out = nc.dram_tensor("i_out", [TPC, K], dt.uint32, kind="ExternalOutput")

    def bc_mid(ap8, n=8):
        # [128, m] -> [128, n(bcast), m]
        return bass.AP(ap8.tensor, ap8.offset, [list(ap8.ap[0]), [0, n], list(ap8.ap[1])])

    with tile.TileContext(nc) as tc:
        with (
            tc.tile_pool(name="wpool", bufs=1) as wpool,
            tc.tile_pool(name="xpool", bufs=6) as xpool,
            tc.tile_pool(name="lpool", bufs=12) as lpool,
            tc.tile_pool(name="qpool", bufs=9) as qpool,
            tc.tile_pool(name="ppool", bufs=8, space="PSUM") as ppool,
            tc.tile_pool(name="spool", bufs=5) as spool,
            tc.tile_pool(name="tpool", bufs=5) as tpool,
            tc.tile_pool(name="opool", bufs=1) as opool,
            tc.tile_pool(name="dpool", bufs=16) as dpool,
        ):
            # ---------------- input DMAs, in delivery-priority order -------
            w16q = [wpool.tile([128, KQ * E], dt.float16, tag=f"w16q{q}", name=f"w16q{q}") for q in range(4)]
            w16_flat = w16_d[:].rearrange("p k e -> p (k e)")
            xts, xlts = [None] * NTILES, [None] * NTILES
            x8ts = [None] * NTILES

            def dma_x(i):
                xt = xpool.tile([128, KCH * 128], dt.float16, tag="x")
                nc.sync.dma_start(xt[:], xh_d[i].rearrange("p k t -> p (k t)"))
                xts[i] = xt

            def dma_xl(i):
                lt = lpool.tile([128, KCH * 128], dt.float8e4, tag="xl")
                nc.sync.dma_start(lt[:], xl_d[i].rearrange("p k t -> p (k t)"))
                xlts[i] = lt
                qt = qpool.tile([128, KCH * 128], dt.float8e4, tag="x8")
                nc.sync.dma_start(qt[:], x8_d[i].rearrange("p k t -> p (k t)"))
                x8ts[i] = qt

            nc.sync.dma_start(w16q[0][:], w16_flat[:, 0:KQ * E])
            dma_x(0)
            nc.sync.dma_start(w16q[1][:], w16_flat[:, KQ * E:2 * KQ * E])
            nc.sync.dma_start(w16q[2][:], w16_flat[:, 2 * KQ * E:3 * KQ * E])
            nc.sync.dma_start(w16q[3][:], w16_flat[:, 3 * KQ * E:4 * KQ * E])
            wl8 = wpool.tile([128, KCH * E], dt.float8e4, tag="wl8")
            wl_flat = wl_d[:].rearrange("p k e -> p (k e)")
            nc.sync.dma_start(wl8[:, 0:KCH * E // 2], wl_flat[:, 0:KCH * E // 2])
            nc.sync.dma_start(wl8[:, KCH * E // 2:], wl_flat[:, KCH * E // 2:])
            dma_xl(0)
            btab = wpool.tile([128, E], dt.float32, tag="btab")
            nc.sync.dma_start(btab[:], btab_d[:])
            for i in range(1, NTILES):
                dma_x(i)
                dma_xl(i)

            # ---------------- PE warm-up (no data deps) --------------------
            junk = wpool.tile([128, 384], dt.float16, tag="junk")
            nc.vector.memset(junk[:], 0.0)
            jp = ppool.tile([128, E], dt.float32, tag="ps")
            for _ in range(NWARM):
                nc.tensor.matmul(jp[:], lhsT=junk[:, 0:128], rhs=junk[:, 128:384],
                                 start=True, stop=True)

            # ---------------- on-chip fp8 derivations (DVE) ----------------
            # w8 = fp8(w * 2^7) = fp8(w16 * 2^-12), one-time
            w8d = wpool.tile([128, KCH * E], dt.float8e4, tag="w8d")
            for q in range(4):
                nc.vector.tensor_scalar(w8d[:, q * KQ * E:(q + 1) * KQ * E],
                                        w16q[q][:], 2.0 ** -12, None, op0=Alu.mult)

            psums = [None] * NTILES
            w8s = [None] * NTILES
            out_w = opool.tile([128, NTILES * K], dt.float32, tag="ow")
            dtile = opool.tile([128, NTILES], dt.float32, tag="dens")
            rtile = opool.tile([128, NTILES], dt.float32, tag="recs")
            out_i = opool.tile([128, NTILES * K], dt.uint32, tag="oi")

            def main_half(i, h):
                if h == 0:
                    psums[i] = ppool.tile([128, E], dt.float32, tag="ps", name="ps")
                ps = psums[i]
                x = xts[i]
                for k in range(h * KCH // 2, (h + 1) * KCH // 2):
                    nc.tensor.matmul(ps[:],
                                     lhsT=x[:, k * 128:(k + 1) * 128],
                                     rhs=w16q[k // KQ][:, (k % KQ) * E:(k % KQ + 1) * E],
                                     start=(k == 0), stop=False)

            def main_pass(i):
                main_half(i, 0)
                main_half(i, 1)

            def corr_pass(i):
                ps = psums[i]
                u3 = x8ts[i][:].rearrange("p (k t) -> p k t", k=KCH)
                l3 = xlts[i][:].rearrange("p (k t) -> p k t", k=KCH)
                va = wl8[:].rearrange("p (k e) -> p k e", k=KCH)
                vb = w8d[:].rearrange("p (k e) -> p k e", k=KCH)
                for j in range(KCH // 2):
                    nc.tensor.matmul(ps[:], lhsT=u3[:, 2 * j:2 * j + 2, :],
                                     rhs=va[:, 2 * j:2 * j + 2, :],
                                     start=False, stop=False, perf_mode=DR)
                for j in range(KCH // 2):
                    nc.tensor.matmul(ps[:], lhsT=l3[:, 2 * j:2 * j + 2, :],
                                     rhs=vb[:, 2 * j:2 * j + 2, :],
                                     start=False, stop=(j == KCH // 2 - 1), perf_mode=DR)

            def epilogue(i):
                ps = psums[i]
                scores = spool.tile([128, E], dt.float32, tag="scores")
                nc.scalar.activation(scores[:], ps[:], Act.Sigmoid, scale=SCALE)
                sr = spool.tile([128, E], dt.float32, tag="sr")
                nc.vector.tensor_tensor(sr[:], scores[:], btab[:], Alu.add)
                sr3 = sr[:].rearrange("p (g e) -> p g e", g=G)

                top1 = tpool.tile([128, G], dt.float32, tag="top1")
                nc.vector.tensor_reduce(top1[:], sr3, axis=Ax.X, op=Alu.max)
                mr2 = spool.tile([128, E], dt.float32, tag="mr2")
                nc.vector.match_replace(mr2[:], in_to_replace=top1[:], in_values=sr[:], imm_value=NEG)
                top2 = tpool.tile([128, G], dt.float32, tag="top2")
                nc.vector.tensor_reduce(top2[:], mr2[:].rearrange("p (g e) -> p g e", g=G), axis=Ax.X, op=Alu.max)
                gs_t = tpool.tile([128, G], dt.float32, tag="gs")
                nc.vector.tensor_tensor(gs_t[:], top1[:], top2[:], Alu.add)
                g8 = tpool.tile([128, 8], dt.float32, tag="g8")
                nc.vector.max(out=g8[:], in_=gs_t[:])
                # inv = 1e4 for groups strictly below the 4th-largest group score
                inv = tpool.tile([128, G], dt.float32, tag="inv")
                nc.vector.tensor_scalar(inv[:], gs_t[:], g8[:, 3:4], -NEG, op0=Alu.is_lt, op1=Alu.mult)
                nc.vector.tensor_tensor(sr3, sr3, inv[:].to_broadcast([128, G, GS]), Alu.subtract)

                vals8 = tpool.tile([128, K], dt.float32, tag="vals8")
                nc.vector.max(out=vals8[:], in_=sr[:])
                oi_c = out_i[:, i * K:(i + 1) * K]
                nc.vector.max_index(out=oi_c, in_max=vals8[:], in_values=sr[:])

                # sel = scores at the selected positions (threshold at the 8th value)
                sel = spool.tile([128, E], dt.float32, tag="sel")
                nc.vector.scalar_tensor_tensor(sel[:], in0=sr[:], scalar=vals8[:, 7:8],
                                               in1=scores[:], op0=Alu.is_ge, op1=Alu.mult)
                svals8 = tpool.tile([128, K], dt.float32, tag="svals8")
                nc.vector.max(out=svals8[:], in_=sel[:])
                sidx8 = tpool.tile([128, K], dt.uint32, tag="sidx8")
                nc.vector.max_index(out=sidx8[:], in_max=svals8[:], in_values=sel[:])

                # reorder svals8 (score order) into routing order: K x K match
                eq = tpool.tile([128, K * K], dt.float32, tag="eq")
                eq3 = eq[:].rearrange("p (k j) -> p k j", k=K)
                nc.vector.tensor_tensor(eq3, oi_c.to_broadcast([128, K, K]), bc_mid(sidx8[:]), Alu.is_equal)
                nc.gpsimd.tensor_tensor(eq3, eq3, bc_mid(svals8[:]), Alu.mult)
                w84 = tpool.tile([128, K * 4], dt.float32, tag="w84")
                w843 = w84[:].rearrange("p (k j) -> p k j", k=K)
                nc.gpsimd.tensor_tensor(w843, eq3[:, :, 0:4], eq3[:, :, 4:8], Alu.add)
                w82 = tpool.tile([128, K * 2], dt.float32, tag="w82")
                w823 = w82[:].rearrange("p (k j) -> p k j", k=K)
                nc.gpsimd.tensor_tensor(w823, w843[:, :, 0:2], w843[:, :, 2:4], Alu.add)
                w8 = dpool.tile([128, K], dt.float32, tag="w8")
                nc.gpsimd.tensor_tensor(w8[:].rearrange("p (k o) -> p k o", o=1),
                                        w823[:, :, 0:1], w823[:, :, 1:2], Alu.add)

                s4 = tpool.tile([128, 4], dt.float32, tag="s4")
                nc.gpsimd.tensor_tensor(s4[:], w8[:, 0:4], w8[:, 4:8], Alu.add)
                s2 = tpool.tile([128, 2], dt.float32, tag="s2")
                nc.gpsimd.tensor_tensor(s2[:], s4[:, 0:2], s4[:, 2:4], Alu.add)
                dc = dtile[:, i:i + 1]
                nc.gpsimd.tensor_tensor(dc, s2[:, 0:1], s2[:, 1:2], Alu.add)
                nc.gpsimd.tensor_scalar(dc, dc, 1.0 / 2.5, None, op0=Alu.mult)
                w8s[i] = w8

            # ---------------- schedule ------------------------------------
            # Per-tile [main, correction] pairs, corrections starting as early
            # as DMA allows so the (DVE-bound) epilogues get the widest
            # possible window. All normalize/scale finishes run at the very
            # end so the DVE stream never blocks mid-run on the Pool trees.
            for i in range(NTILES):
                main_pass(i)
                corr_pass(i)
                epilogue(i)
            nc.vector.reciprocal(rtile[:], dtile[:])
            for t in range(NTILES):
                nc.scalar.activation(out_w[:, t * K:(t + 1) * K], w8s[t][:], Act.Copy,
                                     scale=rtile[:, t:t + 1])
            for g in range(2):
                wo = w_out[:].rearrange("(i p) k -> p i k", p=128)[:, 8 * g:8 * g + 8, :]
                io = i_out[:].rearrange("(i p) k -> p i k", p=128)[:, 8 * g:8 * g + 8, :]
                nc.sync.dma_start(wo, out_w[:, 64 * g:64 * (g + 1)].rearrange("p (i k) -> p i k", i=8))
                nc.sync.dma_start(io, out_i[:, 64 * g:64 * (g + 1)].rearrange("p (i k) -> p i k", i=8))

    nc.compile()
    return nc


def _prep(hidden_states, weight, expert_bias):
    import ml_dtypes
    f8 = ml_dtypes.float8_e4m3
    x = np.ascontiguousarray(hidden_states, dtype=np.float32)
    w = np.ascontiguousarray(weight, dtype=np.float32)

    w16s = (w * 2.0 ** 19).astype(np.float16)            # shipped main weights
    wl = w - w16s.astype(np.float32) * 2.0 ** -19
    wl8 = (wl * 2.0 ** 19).astype(f8)                    # scaled residual of w
    w16_l = np.ascontiguousarray(w16s.reshape(E, KCH, 128).transpose(2, 1, 0))
    wl8_l = np.ascontiguousarray(wl8.reshape(E, KCH, 128).transpose(2, 1, 0))
    btab = np.ascontiguousarray(np.broadcast_to(expert_bias.astype(np.float32), (128, E)))

    in_maps = []
    for c in range(NCORES):
        xs = x[c * TPC:(c + 1) * TPC]
        xh16 = xs.astype(np.float16)
        xl = xs - xh16.astype(np.float32)
        xl8 = (xl * 2.0 ** 12).astype(f8)
        x8 = xs.astype(f8)
        xh_l = np.ascontiguousarray(xh16.reshape(NTILES, 128, KCH, 128).transpose(0, 3, 2, 1))
        xl8_l = np.ascontiguousarray(xl8.reshape(NTILES, 128, KCH, 128).transpose(0, 3, 2, 1))
        x8_l = np.ascontiguousarray(x8.reshape(NTILES, 128, KCH, 128).transpose(0, 3, 2, 1))
        in_maps.append({"xh": xh_l, "xl": xl8_l, "x8": x8_l, "w16": w16_l, "wl": wl8_l, "btab": btab})
    return in_maps


def kernel(hidden_states, weight, expert_bias, _trace=False):
    from concourse.bass_utils import run_bass_kernel_spmd

    if "nc" not in _cache:
        _cache["nc"] = _build()
    nc = _cache["nc"]
    in_maps = _prep(hidden_states, weight, expert_bias)
    res = run_bass_kernel_spmd(nc, in_maps, core_ids=list(range(NCORES)), trace=_trace)
    _cache["last_results"] = res
    w = np.concatenate([res.results[c]["w_out"] for c in range(NCORES)], axis=0)
    idx = np.concatenate([res.results[c]["i_out"] for c in range(NCORES)], axis=0)
    return w.astype(np.float32), idx.astype(np.int32)


# revision 24
# speedup vs baseline: 1.0321x; 1.0166x over previous
"""MoE gate (LLaDA2) routing kernel for 8 Trainium2 NeuronCores.

Token-parallel over 8 cores (2048 tokens/core, 16 tiles of 128). Router GEMM
runs as an fp16 main pass (xh @ w16*2^19) plus a DoubleRow fp8 correction
(x8@wl8*2^19 + xl8*2^12 @ w8*2^7), all accumulating into ONE PSUM bank per
tile at 2^19 scale, so the sigmoid reads PSUM directly with scale=2^-19 and
no combine/copy ops are needed. w8 (= fp8(w*2^7)) is derived on-chip once.
PE schedule: warm-up junk matmuls (HAM clock-gate release) -> 8 main passes
run ahead of their corrections (8 open PSUM banks) -> D/m interleave ->
correction drain. Grouped top-8 routing epilogue: all 256-wide work on DVE
(GPSIMD is ~8ns/elem/partition -- only <=64-elem ops go to Pool), ACT does
the sigmoid, Pool does the K x K reorder trees + normalize_recip.
"""
import sys
for p in ("/opt/trn_rl_repo", "/root/.axon_site/_ro/trn_rl_repo"):
    if p not in sys.path:
        sys.path.append(p)

import numpy as np

T, H, E = 16384, 4096, 256
NCORES = 8
TPC = T // NCORES          # tokens per core: 2048
NTILES = TPC // 128        # 16 row tiles
KCH = H // 128             # 32 contraction chunks
KQ = KCH // 4              # chunks per w16 quarter
G = 8                      # expert groups
GS = E // G                # 32 experts/group
K = 8                      # top-k
BIG = 2.0 ** 100
NEG = -1.0e4
SCALE = 2.0 ** -19         # undo the 2^19 logit scaling at sigmoid time
NWARM = 30                 # junk matmuls to release the PE HAM clock gate
AHEAD = 8                  # main passes run before the first correction pass

_cache = {}


def _build():
    import concourse.bacc as bacc
    import concourse.bass as bass
    import concourse.mybir as mybir
    from concourse import tile

    dt = mybir.dt
    Alu = mybir.AluOpType
    Act = mybir.ActivationFunctionType
    Ax = mybir.AxisListType
    DR = mybir.MatmulPerfMode.DoubleRow

    nc = bacc.Bacc("TRN2", target_bir_lowering=False, debug=False,
                   num_devices=NCORES)

    xh_d = nc.dram_tensor("xh", [NTILES, 128, KCH, 128], dt.float16, kind="ExternalInput")
    xl_d = nc.dram_tensor("xl", [NTILES, 128, KCH, 128], dt.float8e4, kind="ExternalInput")
    x8_d = nc.dram_tensor("x8", [NTILES, 128, KCH, 128], dt.float8e4, kind="ExternalInput")
    w16_d = nc.dram_tensor("w16", [128, KCH, E], dt.float16, kind="ExternalInput")
    wl_d = nc.dram_tensor("wl", [128, KCH, E], dt.float8e4, kind="ExternalInput")
    btab_d = nc.dram_tensor("btab", [128, E], dt.float32, kind="ExternalInput")
    w_out = nc.dram_tensor("w_out", [TPC, K], dt.float32, kind="ExternalOutput")
    i_out = nc.dram_tensor("i_out", [TPC, K], dt.uint32, kind="ExternalOutput")

    def bc_mid(ap8, n=8):
        # [128, m] -> [128, n(bcast), m]
        return bass.AP(ap8.tensor, ap8.offset, [list(ap8.ap[0]), [0, n], list(ap8.ap[1])])

    with tile.TileContext(nc) as tc:
        with (
            tc.tile_pool(name="wpool", bufs=1) as wpool,
            tc.tile_pool(name="xpool", bufs=6) as xpool,
            tc.tile_pool(name="lpool", bufs=12) as lpool,
            tc.tile_pool(name="qpool", bufs=9) as qpool,
            tc.tile_pool(name="ppool", bufs=8, space="PSUM") as ppool,
            tc.tile_pool(name="spool", bufs=5) as spool,
            tc.tile_pool(name="tpool", bufs=5) as tpool,
            tc.tile_pool(name="opool", bufs=1) as opool,
            tc.tile_pool(name="dpool", bufs=16) as dpool,
        ):
            # ---------------- input DMAs, in delivery-priority order -------
            w16q = [wpool.tile([128, KQ * E], dt.float16, tag=f"w16q{q}", name=f"w16q{q}") for q in range(4)]
            w16_flat = w16_d[:].rearrange("p k e -> p (k e)")
            xts, xlts = [None] * NTILES, [None] * NTILES
            x8ts = [None] * NTILES

            def dma_x(i):
                xt = xpool.tile([128, KCH * 128], dt.float16, tag="x")
                nc.sync.dma_start(xt[:], xh_d[i].rearrange("p k t -> p (k t)"))
                xts[i] = xt

            def dma_xl(i):
                lt = lpool.tile([128, KCH * 128], dt.float8e4, tag="xl")
                nc.sync.dma_start(lt[:], xl_d[i].rearrange("p k t -> p (k t)"))
                xlts[i] = lt
                qt = qpool.tile([128, KCH * 128], dt.float8e4, tag="x8")
                nc.sync.dma_start(qt[:], x8_d[i].rearrange("p k t -> p (k t)"))
                x8ts[i] = qt

            nc.sync.dma_start(w16q[0][:], w16_flat[:, 0:KQ * E])
            dma_x(0)
            nc.sync.dma_start(w16q[1][:], w16_flat[:, KQ * E:2 * KQ * E])
            nc.sync.dma_start(w16q[2][:], w16_flat[:, 2 * KQ * E:3 * KQ * E])
            nc.sync.dma_start(w16q[3][:], w16_flat[:, 3 * KQ * E:4 * KQ * E])
            wl8 = wpool.tile([128, KCH * E], dt.float8e4, tag="wl8")
            wl_flat = wl_d[:].rearrange("p k e -> p (k e)")
            nc.sync.dma_start(wl8[:, 0:KCH * E // 2], wl_flat[:, 0:KCH * E // 2])
            nc.sync.dma_start(wl8[:, KCH * E // 2:], wl_flat[:, KCH * E // 2:])
            dma_xl(0)
            btab = wpool.tile([128, E], dt.float32, tag="btab")
            nc.sync.dma_start(btab[:], btab_d[:])
            for i in range(1, NTILES):
                dma_x(i)
                dma_xl(i)

            # ---------------- PE warm-up (no data deps) --------------------
            junk = wpool.tile([128, 384], dt.float16, tag="junk")
            nc.vector.memset(junk[:], 0.0)
            jp = ppool.tile([128, E], dt.float32, tag="ps")
            for _ in range(NWARM):
                nc.tensor.matmul(jp[:], lhsT=junk[:, 0:128], rhs=junk[:, 128:384],
                                 start=True, stop=True)

            # ---------------- on-chip fp8 derivations (DVE) ----------------
            # w8 = fp8(w * 2^7) = fp8(w16 * 2^-12), one-time
            w8d = wpool.tile([128, KCH * E], dt.float8e4, tag="w8d")
            for q in range(4):
                nc.vector.tensor_scalar(w8d[:, q * KQ * E:(q + 1) * KQ * E],
                                        w16q[q][:], 2.0 ** -12, None, op0=Alu.mult)

            psums = [None] * NTILES
            w8s = [None] * NTILES
            out_w = opool.tile([128, NTILES * K], dt.float32, tag="ow")
            dtileA = opool.tile([128, 12], dt.float32, tag="densA")
            dtileB = opool.tile([128, 4], dt.float32, tag="densB")
            rtile = opool.tile([128, NTILES], dt.float32, tag="recs")
            out_i = opool.tile([128, NTILES * K], dt.uint32, tag="oi")

            def main_half(i, h):
                if h == 0:
                    psums[i] = ppool.tile([128, E], dt.float32, tag="ps", name="ps")
                ps = psums[i]
                x = xts[i]
                for k in range(h * KCH // 2, (h + 1) * KCH // 2):
                    nc.tensor.matmul(ps[:],
                                     lhsT=x[:, k * 128:(k + 1) * 128],
                                     rhs=w16q[k // KQ][:, (k % KQ) * E:(k % KQ + 1) * E],
                                     start=(k == 0), stop=False)

            def main_pass(i):
                main_half(i, 0)
                main_half(i, 1)

            def corr_pass(i):
                ps = psums[i]
                u3 = x8ts[i][:].rearrange("p (k t) -> p k t", k=KCH)
                l3 = xlts[i][:].rearrange("p (k t) -> p k t", k=KCH)
                va = wl8[:].rearrange("p (k e) -> p k e", k=KCH)
                vb = w8d[:].rearrange("p (k e) -> p k e", k=KCH)
                for j in range(KCH // 2):
                    nc.tensor.matmul(ps[:], lhsT=u3[:, 2 * j:2 * j + 2, :],
                                     rhs=va[:, 2 * j:2 * j + 2, :],
                                     start=False, stop=False, perf_mode=DR)
                for j in range(KCH // 2):
                    nc.tensor.matmul(ps[:], lhsT=l3[:, 2 * j:2 * j + 2, :],
                                     rhs=vb[:, 2 * j:2 * j + 2, :],
                                     start=False, stop=(j == KCH // 2 - 1), perf_mode=DR)

            def epilogue(i):
                ps = psums[i]
                scores = spool.tile([128, E], dt.float32, tag="scores")
                nc.scalar.activation(scores[:], ps[:], Act.Sigmoid, scale=SCALE)
                sr = spool.tile([128, E], dt.float32, tag="sr")
                nc.vector.tensor_tensor(sr[:], scores[:], btab[:], Alu.add)
                sr3 = sr[:].rearrange("p (g e) -> p g e", g=G)

                top1 = tpool.tile([128, G], dt.float32, tag="top1")
                nc.vector.tensor_reduce(top1[:], sr3, axis=Ax.X, op=Alu.max)
                mr2 = spool.tile([128, E], dt.float32, tag="mr2")
                nc.vector.match_replace(mr2[:], in_to_replace=top1[:], in_values=sr[:], imm_value=NEG)
                top2 = tpool.tile([128, G], dt.float32, tag="top2")
                nc.vector.tensor_reduce(top2[:], mr2[:].rearrange("p (g e) -> p g e", g=G), axis=Ax.X, op=Alu.max)
                gs_t = tpool.tile([128, G], dt.float32, tag="gs")
                nc.vector.tensor_tensor(gs_t[:], top1[:], top2[:], Alu.add)
                g8 = tpool.tile([128, 8], dt.float32, tag="g8")
                nc.vector.max(out=g8[:], in_=gs_t[:])
                # inv = 1e4 for groups strictly below the 4th-largest group score
                inv = tpool.tile([128, G], dt.float32, tag="inv")
                nc.vector.tensor_scalar(inv[:], gs_t[:], g8[:, 3:4], -NEG, op0=Alu.is_lt, op1=Alu.mult)
                nc.vector.tensor_tensor(sr3, sr3, inv[:].to_broadcast([128, G, GS]), Alu.subtract)

                vals8 = tpool.tile([128, K], dt.float32, tag="vals8")
                nc.vector.max(out=vals8[:], in_=sr[:])
                oi_c = out_i[:, i * K:(i + 1) * K]
                nc.vector.max_index(out=oi_c, in_max=vals8[:], in_values=sr[:])

                # sel = scores at the selected positions (threshold at the 8th value)
                sel = spool.tile([128, E], dt.float32, tag="sel")
                nc.vector.scalar_tensor_tensor(sel[:], in0=sr[:], scalar=vals8[:, 7:8],
                                               in1=scores[:], op0=Alu.is_ge, op1=Alu.mult)
                svals8 = tpool.tile([128, K], dt.float32, tag="svals8")
                nc.vector.max(out=svals8[:], in_=sel[:])
                sidx8 = tpool.tile([128, K], dt.uint32, tag="sidx8")
                nc.vector.max_index(out=sidx8[:], in_max=svals8[:], in_values=sel[:])

                # reorder svals8 (score order) into routing order: K x K match
                eq = tpool.tile([128, K * K], dt.float32, tag="eq")
                eq3 = eq[:].rearrange("p (k j) -> p k j", k=K)
                nc.vector.tensor_tensor(eq3, oi_c.to_broadcast([128, K, K]), bc_mid(sidx8[:]), Alu.is_equal)
                # the K x K trees run on the idle Pool engine, except the last
                # tile where the faster DVE shortens the kernel tail
                eng = nc.vector if i == NTILES - 1 else nc.gpsimd
                eng.tensor_tensor(eq3, eq3, bc_mid(svals8[:]), Alu.mult)
                w84 = tpool.tile([128, K * 4], dt.float32, tag="w84")
                w843 = w84[:].rearrange("p (k j) -> p k j", k=K)
                eng.tensor_tensor(w843, eq3[:, :, 0:4], eq3[:, :, 4:8], Alu.add)
                w82 = tpool.tile([128, K * 2], dt.float32, tag="w82")
                w823 = w82[:].rearrange("p (k j) -> p k j", k=K)
                eng.tensor_tensor(w823, w843[:, :, 0:2], w843[:, :, 2:4], Alu.add)
                w8 = dpool.tile([128, K], dt.float32, tag="w8")
                eng.tensor_tensor(w8[:].rearrange("p (k o) -> p k o", o=1),
                                  w823[:, :, 0:1], w823[:, :, 1:2], Alu.add)

                s4 = tpool.tile([128, 4], dt.float32, tag="s4")
                eng.tensor_tensor(s4[:], w8[:, 0:4], w8[:, 4:8], Alu.add)
                s2 = tpool.tile([128, 2], dt.float32, tag="s2")
                eng.tensor_tensor(s2[:], s4[:, 0:2], s4[:, 2:4], Alu.add)
                dc = dtileA[:, i:i + 1] if i < 12 else dtileB[:, i - 12:i - 11]
                eng.tensor_tensor(dc, s2[:, 0:1], s2[:, 1:2], Alu.add)
                eng.tensor_scalar(dc, dc, 1.0 / 2.5, None, op0=Alu.mult)
                w8s[i] = w8

            # ---------------- schedule ------------------------------------
            # Per-tile [main, correction] pairs, corrections starting as early
            # as DMA allows so the (DVE-bound) epilogues get the widest
            # possible window. All normalize/scale finishes run at the very
            # end so the DVE stream never blocks mid-run on the Pool trees.
            for i in range(NTILES):
                main_pass(i)
                corr_pass(i)
                epilogue(i)
            nc.vector.reciprocal(rtile[:, 0:12], dtileA[:])
            for t in range(12):
                nc.scalar.activation(out_w[:, t * K:(t + 1) * K], w8s[t][:], Act.Copy,
                                     scale=rtile[:, t:t + 1])
            wo = w_out[:].rearrange("(i p) k -> p i k", p=128)[:, 0:12, :]
            io = i_out[:].rearrange("(i p) k -> p i k", p=128)[:, 0:12, :]
            nc.sync.dma_start(wo, out_w[:, 0:96].rearrange("p (i k) -> p i k", i=12))
            nc.sync.dma_start(io, out_i[:, 0:96].rearrange("p (i k) -> p i k", i=12))
            nc.vector.reciprocal(rtile[:, 12:16], dtileB[:])
            for t in range(12, NTILES):
                nc.scalar.activation(out_w[:, t * K:(t + 1) * K], w8s[t][:], Act.Copy,
                                     scale=rtile[:, t:t + 1])
            wo = w_out[:].rearrange("(i p) k -> p i k", p=128)[:, 12:16, :]
            io = i_out[:].rearrange("(i p) k -> p i k", p=128)[:, 12:16, :]
            nc.sync.dma_start(wo, out_w[:, 96:128].rearrange("p (i k) -> p i k", i=4))
            nc.sync.dma_start(io, out_i[:, 96:128].rearrange("p (i k) -> p i k", i=4))

    nc.compile()
    return nc


def _prep(hidden_states, weight, expert_bias):
    import ml_dtypes
    f8 = ml_dtypes.float8_e4m3
    x = np.ascontiguousarray(hidden_states, dtype=np.float32)
    w = np.ascontiguousarray(weight, dtype=np.float32)

    w16s = (w * 2.0 ** 19).astype(np.float16)            # shipped main weights
    wl = w - w16s.astype(np.float32) * 2.0 ** -19
    wl8 = (wl * 2.0 ** 19).astype(f8)                    # scaled residual of w
    w16_l = np.ascontiguousarray(w16s.reshape(E, KCH, 128).transpose(2, 1, 0))
    wl8_l = np.ascontiguousarray(wl8.reshape(E, KCH, 128).transpose(2, 1, 0))
    btab = np.ascontiguousarray(np.broadcast_to(expert_bias.astype(np.float32), (128, E)))

    in_maps = []
    for c in range(NCORES):
        xs = x[c * TPC:(c + 1) * TPC]
        xh16 = xs.astype(np.float16)
        xl = xs - xh16.astype(np.float32)
        xl8 = (xl * 2.0 ** 12).astype(f8)
        x8 = xs.astype(f8)
        xh_l = np.ascontiguousarray(xh16.reshape(NTILES, 128, KCH, 128).transpose(0, 3, 2, 1))
        xl8_l = np.ascontiguousarray(xl8.reshape(NTILES, 128, KCH, 128).transpose(0, 3, 2, 1))
        x8_l = np.ascontiguousarray(x8.reshape(NTILES, 128, KCH, 128).transpose(0, 3, 2, 1))
        in_maps.append({"xh": xh_l, "xl": xl8_l, "x8": x8_l, "w16": w16_l, "wl": wl8_l, "btab": btab})
    return in_maps


def kernel(hidden_states, weight, expert_bias, _trace=False):
    from concourse.bass_utils import run_bass_kernel_spmd

    if "nc" not in _cache:
        _cache["nc"] = _build()
    nc = _cache["nc"]
    in_maps = _prep(hidden_states, weight, expert_bias)
    res = run_bass_kernel_spmd(nc, in_maps, core_ids=list(range(NCORES)), trace=_trace)
    _cache["last_results"] = res
    w = np.concatenate([res.results[c]["w_out"] for c in range(NCORES)], axis=0)
    idx = np.concatenate([res.results[c]["i_out"] for c in range(NCORES)], axis=0)
    return w.astype(np.float32), idx.astype(np.int32)


# revision 27
# speedup vs baseline: 1.0330x; 1.0008x over previous
"""MoE gate (LLaDA2) routing kernel for 8 Trainium2 NeuronCores.

Token-parallel over 8 cores (2048 tokens/core, 16 tiles of 128). Router GEMM
runs as an fp16 main pass (xh @ w16*2^19) plus a DoubleRow fp8 correction
(x8@wl8*2^19 + xl8*2^12 @ w8*2^7), all accumulating into ONE PSUM bank per
tile at 2^19 scale, so the sigmoid reads PSUM directly with scale=2^-19 and
no combine/copy ops are needed. w8 (= fp8(w*2^7)) is derived on-chip once.
PE schedule: warm-up junk matmuls (HAM clock-gate release), then per-tile
[main, correction] pairs with corrections starting as early as DMA allows,
giving the DVE-bound epilogues the widest window. Grouped top-8 routing
epilogue: all 256-wide work on DVE (GPSIMD is ~8ns/elem/partition -- only
<=64-elem ops go to Pool; its custom-lib ops cost a ~7us library swap, so
none are used), ACT does sigmoid + final scale-by-reciprocal copies; the
normalization reciprocals run batched at the end (dens split 12/4 so the
first 12 tiles' outputs stream out early), and the last tile's reorder
trees run on DVE to shorten the kernel tail.
"""
import sys
for p in ("/opt/trn_rl_repo", "/root/.axon_site/_ro/trn_rl_repo"):
    if p not in sys.path:
        sys.path.append(p)

import numpy as np

T, H, E = 16384, 4096, 256
NCORES = 8
TPC = T // NCORES          # tokens per core: 2048
NTILES = TPC // 128        # 16 row tiles
KCH = H // 128             # 32 contraction chunks
KQ = KCH // 4              # chunks per w16 quarter
G = 8                      # expert groups
GS = E // G                # 32 experts/group
K = 8                      # top-k
BIG = 2.0 ** 100
NEG = -1.0e4
SCALE = 2.0 ** -19         # undo the 2^19 logit scaling at sigmoid time
NWARM = 30                 # junk matmuls to release the PE HAM clock gate
AHEAD = 8                  # main passes run before the first correction pass

_cache = {}


def _build():
    import concourse.bacc as bacc
    import concourse.bass as bass
    import concourse.mybir as mybir
    from concourse import tile

    dt = mybir.dt
    Alu = mybir.AluOpType
    Act = mybir.ActivationFunctionType
    Ax = mybir.AxisListType
    DR = mybir.MatmulPerfMode.DoubleRow

    nc = bacc.Bacc("TRN2", target_bir_lowering=False, debug=False,
                   num_devices=NCORES)

    xh_d = nc.dram_tensor("xh", [NTILES, 128, KCH, 128], dt.float16, kind="ExternalInput")
    xl_d = nc.dram_tensor("xl", [NTILES, 128, KCH, 128], dt.float8e4, kind="ExternalInput")
    x8_d = nc.dram_tensor("x8", [NTILES, 128, KCH, 128], dt.float8e4, kind="ExternalInput")
    w16_d = nc.dram_tensor("w16", [128, KCH, E], dt.float16, kind="ExternalInput")
    wl_d = nc.dram_tensor("wl", [128, KCH, E], dt.float8e4, kind="ExternalInput")
    btab_d = nc.dram_tensor("btab", [128, E], dt.float32, kind="ExternalInput")
    w_out = nc.dram_tensor("w_out", [TPC, K], dt.float32, kind="ExternalOutput")
    i_out = nc.dram_tensor("i_out", [TPC, K], dt.uint32, kind="ExternalOutput")

    def bc_mid(ap8, n=8):
        # [128, m] -> [128, n(bcast), m]
        return bass.AP(ap8.tensor, ap8.offset, [list(ap8.ap[0]), [0, n], list(ap8.ap[1])])

    with tile.TileContext(nc) as tc:
        with (
            tc.tile_pool(name="wpool", bufs=1) as wpool,
            tc.tile_pool(name="xpool", bufs=6) as xpool,
            tc.tile_pool(name="lpool", bufs=12) as lpool,
            tc.tile_pool(name="qpool", bufs=9) as qpool,
            tc.tile_pool(name="ppool", bufs=8, space="PSUM") as ppool,
            tc.tile_pool(name="spool", bufs=5) as spool,
            tc.tile_pool(name="tpool", bufs=5) as tpool,
            tc.tile_pool(name="opool", bufs=1) as opool,
            tc.tile_pool(name="dpool", bufs=16) as dpool,
        ):
            # ---------------- input DMAs, in delivery-priority order -------
            w16q = [wpool.tile([128, KQ * E], dt.float16, tag=f"w16q{q}", name=f"w16q{q}") for q in range(4)]
            w16_flat = w16_d[:].rearrange("p k e -> p (k e)")
            xts, xlts = [None] * NTILES, [None] * NTILES
            x8ts = [None] * NTILES

            def dma_x(i):
                xt = xpool.tile([128, KCH * 128], dt.float16, tag="x")
                nc.sync.dma_start(xt[:], xh_d[i].rearrange("p k t -> p (k t)"))
                xts[i] = xt

            def dma_xl(i):
                lt = lpool.tile([128, KCH * 128], dt.float8e4, tag="xl")
                nc.sync.dma_start(lt[:], xl_d[i].rearrange("p k t -> p (k t)"))
                xlts[i] = lt
                qt = qpool.tile([128, KCH * 128], dt.float8e4, tag="x8")
                nc.sync.dma_start(qt[:], x8_d[i].rearrange("p k t -> p (k t)"))
                x8ts[i] = qt

            nc.sync.dma_start(w16q[0][:], w16_flat[:, 0:KQ * E])
            dma_x(0)
            nc.sync.dma_start(w16q[1][:], w16_flat[:, KQ * E:2 * KQ * E])
            nc.sync.dma_start(w16q[2][:], w16_flat[:, 2 * KQ * E:3 * KQ * E])
            nc.sync.dma_start(w16q[3][:], w16_flat[:, 3 * KQ * E:4 * KQ * E])
            wl_flat = wl_d[:].rearrange("p k e -> p (k e)")
            dma_xl(0)
            wl8a = wpool.tile([128, KCH * E // 2], dt.float8e4, tag="wl8a")
            nc.sync.dma_start(wl8a[:], wl_flat[:, 0:KCH * E // 2])
            wl8b = wpool.tile([128, KCH * E // 2], dt.float8e4, tag="wl8b")
            nc.sync.dma_start(wl8b[:], wl_flat[:, KCH * E // 2:])
            btab = wpool.tile([128, E], dt.float32, tag="btab")
            nc.sync.dma_start(btab[:], btab_d[:])
            for i in range(1, NTILES):
                dma_x(i)
                dma_xl(i)

            # ---------------- PE warm-up (no data deps) --------------------
            junk = wpool.tile([128, 384], dt.float16, tag="junk")
            nc.vector.memset(junk[:], 0.0)
            jp = ppool.tile([128, E], dt.float32, tag="ps")
            for _ in range(NWARM):
                nc.tensor.matmul(jp[:], lhsT=junk[:, 0:128], rhs=junk[:, 128:384],
                                 start=True, stop=True)

            # ---------------- on-chip fp8 derivations (DVE) ----------------
            # w8 = fp8(w * 2^7) = fp8(w16 * 2^-12), one-time
            w8d = wpool.tile([128, KCH * E], dt.float8e4, tag="w8d")
            for q in range(4):
                nc.vector.tensor_scalar(w8d[:, q * KQ * E:(q + 1) * KQ * E],
                                        w16q[q][:], 2.0 ** -12, None, op0=Alu.mult)

            psums = [None] * NTILES
            w8s = [None] * NTILES
            out_w = opool.tile([128, NTILES * K], dt.float32, tag="ow")
            dtileA = opool.tile([128, 12], dt.float32, tag="densA")
            dtileB = opool.tile([128, 4], dt.float32, tag="densB")
            rtile = opool.tile([128, NTILES], dt.float32, tag="recs")
            out_i = opool.tile([128, NTILES * K], dt.uint32, tag="oi")

            def main_half(i, h):
                if h == 0:
                    psums[i] = ppool.tile([128, E], dt.float32, tag="ps", name="ps")
                ps = psums[i]
                x = xts[i]
                for k in range(h * KCH // 2, (h + 1) * KCH // 2):
                    nc.tensor.matmul(ps[:],
                                     lhsT=x[:, k * 128:(k + 1) * 128],
                                     rhs=w16q[k // KQ][:, (k % KQ) * E:(k % KQ + 1) * E],
                                     start=(k == 0), stop=False)

            def main_pass(i):
                main_half(i, 0)
                main_half(i, 1)

            def corr_pass(i):
                # xl8 x w8d half first: its operands (xl8_i + derived w8d)
                # land before wl8, so the first correction starts sooner
                ps = psums[i]
                u3 = x8ts[i][:].rearrange("p (k t) -> p k t", k=KCH)
                l3 = xlts[i][:].rearrange("p (k t) -> p k t", k=KCH)
                va = wl8a[:].rearrange("p (k e) -> p k e", k=KCH // 2)
                vc = wl8b[:].rearrange("p (k e) -> p k e", k=KCH // 2)
                vb = w8d[:].rearrange("p (k e) -> p k e", k=KCH)
                for j in range(KCH // 2):
                    nc.tensor.matmul(ps[:], lhsT=l3[:, 2 * j:2 * j + 2, :],
                                     rhs=vb[:, 2 * j:2 * j + 2, :],
                                     start=False, stop=False, perf_mode=DR)
                for j in range(KCH // 4):
                    nc.tensor.matmul(ps[:], lhsT=u3[:, 2 * j:2 * j + 2, :],
                                     rhs=va[:, 2 * j:2 * j + 2, :],
                                     start=False, stop=False, perf_mode=DR)
                for j in range(KCH // 4):
                    nc.tensor.matmul(ps[:], lhsT=u3[:, 16 + 2 * j:16 + 2 * j + 2, :],
                                     rhs=vc[:, 2 * j:2 * j + 2, :],
                                     start=False, stop=(j == KCH // 4 - 1), perf_mode=DR)

            def epilogue(i):
                ps = psums[i]
                scores = spool.tile([128, E], dt.float32, tag="scores")
                nc.scalar.activation(scores[:], ps[:], Act.Sigmoid, scale=SCALE)
                sr = spool.tile([128, E], dt.float32, tag="sr")
                nc.vector.tensor_tensor(sr[:], scores[:], btab[:], Alu.add)
                sr3 = sr[:].rearrange("p (g e) -> p g e", g=G)

                top1 = tpool.tile([128, G], dt.float32, tag="top1")
                nc.vector.tensor_reduce(top1[:], sr3, axis=Ax.X, op=Alu.max)
                mr2 = spool.tile([128, E], dt.float32, tag="mr2")
                nc.vector.match_replace(mr2[:], in_to_replace=top1[:], in_values=sr[:], imm_value=NEG)
                top2 = tpool.tile([128, G], dt.float32, tag="top2")
                nc.vector.tensor_reduce(top2[:], mr2[:].rearrange("p (g e) -> p g e", g=G), axis=Ax.X, op=Alu.max)
                gs_t = tpool.tile([128, G], dt.float32, tag="gs")
                nc.vector.tensor_tensor(gs_t[:], top1[:], top2[:], Alu.add)
                g8 = tpool.tile([128, 8], dt.float32, tag="g8")
                nc.vector.max(out=g8[:], in_=gs_t[:])
                # inv = 1e4 for groups strictly below the 4th-largest group score
                inv = tpool.tile([128, G], dt.float32, tag="inv")
                nc.vector.tensor_scalar(inv[:], gs_t[:], g8[:, 3:4], -NEG, op0=Alu.is_lt, op1=Alu.mult)
                nc.vector.tensor_tensor(sr3, sr3, inv[:].to_broadcast([128, G, GS]), Alu.subtract)

                vals8 = tpool.tile([128, K], dt.float32, tag="vals8")
                nc.vector.max(out=vals8[:], in_=sr[:])
                oi_c = out_i[:, i * K:(i + 1) * K]
                nc.vector.max_index(out=oi_c, in_max=vals8[:], in_values=sr[:])

                # sel = scores at the selected positions (threshold at the 8th value)
                sel = spool.tile([128, E], dt.float32, tag="sel")
                nc.vector.scalar_tensor_tensor(sel[:], in0=sr[:], scalar=vals8[:, 7:8],
                                               in1=scores[:], op0=Alu.is_ge, op1=Alu.mult)
                svals8 = tpool.tile([128, K], dt.float32, tag="svals8")
                nc.vector.max(out=svals8[:], in_=sel[:])
                sidx8 = tpool.tile([128, K], dt.uint32, tag="sidx8")
                nc.vector.max_index(out=sidx8[:], in_max=svals8[:], in_values=sel[:])

                # reorder svals8 (score order) into routing order: K x K match
                eq = tpool.tile([128, K * K], dt.float32, tag="eq")
                eq3 = eq[:].rearrange("p (k j) -> p k j", k=K)
                nc.vector.tensor_tensor(eq3, oi_c.to_broadcast([128, K, K]), bc_mid(sidx8[:]), Alu.is_equal)
                # the K x K trees run on the idle Pool engine, except the last
                # tile where the faster DVE shortens the kernel tail
                eng = nc.vector if i == NTILES - 1 else nc.gpsimd
                eng.tensor_tensor(eq3, eq3, bc_mid(svals8[:]), Alu.mult)
                w84 = tpool.tile([128, K * 4], dt.float32, tag="w84")
                w843 = w84[:].rearrange("p (k j) -> p k j", k=K)
                eng.tensor_tensor(w843, eq3[:, :, 0:4], eq3[:, :, 4:8], Alu.add)
                w82 = tpool.tile([128, K * 2], dt.float32, tag="w82")
                w823 = w82[:].rearrange("p (k j) -> p k j", k=K)
                eng.tensor_tensor(w823, w843[:, :, 0:2], w843[:, :, 2:4], Alu.add)
                w8 = dpool.tile([128, K], dt.float32, tag="w8")
                eng.tensor_tensor(w8[:].rearrange("p (k o) -> p k o", o=1),
                                  w823[:, :, 0:1], w823[:, :, 1:2], Alu.add)

                s4 = tpool.tile([128, 4], dt.float32, tag="s4")
                eng.tensor_tensor(s4[:], w8[:, 0:4], w8[:, 4:8], Alu.add)
                s2 = tpool.tile([128, 2], dt.float32, tag="s2")
                eng.tensor_tensor(s2[:], s4[:, 0:2], s4[:, 2:4], Alu.add)
                dc = dtileA[:, i:i + 1] if i < 12 else dtileB[:, i - 12:i - 11]
                eng.tensor_tensor(dc, s2[:, 0:1], s2[:, 1:2], Alu.add)
                eng.tensor_scalar(dc, dc, 1.0 / 2.5, None, op0=Alu.mult)
                w8s[i] = w8

            # ---------------- schedule ------------------------------------
            # Per-tile [main, correction] pairs, corrections starting as early
            # as DMA allows so the (DVE-bound) epilogues get the widest
            # possible window. All normalize/scale finishes run at the very
            # end so the DVE stream never blocks mid-run on the Pool trees.
            for i in range(NTILES):
                main_pass(i)
                corr_pass(i)
                epilogue(i)
            nc.vector.reciprocal(rtile[:, 0:12], dtileA[:])
            for t in range(12):
                nc.scalar.activation(out_w[:, t * K:(t + 1) * K], w8s[t][:], Act.Copy,
                                     scale=rtile[:, t:t + 1])
            wo = w_out[:].rearrange("(i p) k -> p i k", p=128)[:, 0:12, :]
            io = i_out[:].rearrange("(i p) k -> p i k", p=128)[:, 0:12, :]
            nc.sync.dma_start(wo, out_w[:, 0:96].rearrange("p (i k) -> p i k", i=12))
            nc.sync.dma_start(io, out_i[:, 0:96].rearrange("p (i k) -> p i k", i=12))
            nc.vector.reciprocal(rtile[:, 12:16], dtileB[:])
            for t in range(12, NTILES):
                nc.scalar.activation(out_w[:, t * K:(t + 1) * K], w8s[t][:], Act.Copy,
                                     scale=rtile[:, t:t + 1])
            wo = w_out[:].rearrange("(i p) k -> p i k", p=128)[:, 12:16, :]
            io = i_out[:].rearrange("(i p) k -> p i k", p=128)[:, 12:16, :]
            nc.sync.dma_start(wo, out_w[:, 96:128].rearrange("p (i k) -> p i k", i=4))
            nc.sync.dma_start(io, out_i[:, 96:128].rearrange("p (i k) -> p i k", i=4))

    nc.compile()
    return nc


def _prep(hidden_states, weight, expert_bias):
    import ml_dtypes
    f8 = ml_dtypes.float8_e4m3
    x = np.ascontiguousarray(hidden_states, dtype=np.float32)
    w = np.ascontiguousarray(weight, dtype=np.float32)

    w16s = (w * 2.0 ** 19).astype(np.float16)            # shipped main weights
    wl = w - w16s.astype(np.float32) * 2.0 ** -19
    wl8 = (wl * 2.0 ** 19).astype(f8)                    # scaled residual of w
    w16_l = np.ascontiguousarray(w16s.reshape(E, KCH, 128).transpose(2, 1, 0))
    wl8_l = np.ascontiguousarray(wl8.reshape(E, KCH, 128).transpose(2, 1, 0))
    btab = np.ascontiguousarray(np.broadcast_to(expert_bias.astype(np.float32), (128, E)))

    in_maps = []
    for c in range(NCORES):
        xs = x[c * TPC:(c + 1) * TPC]
        xh16 = xs.astype(np.float16)
        xl = xs - xh16.astype(np.float32)
        xl8 = (xl * 2.0 ** 12).astype(f8)
        x8 = xs.astype(f8)
        xh_l = np.ascontiguousarray(xh16.reshape(NTILES, 128, KCH, 128).transpose(0, 3, 2, 1))
        xl8_l = np.ascontiguousarray(xl8.reshape(NTILES, 128, KCH, 128).transpose(0, 3, 2, 1))
        x8_l = np.ascontiguousarray(x8.reshape(NTILES, 128, KCH, 128).transpose(0, 3, 2, 1))
        in_maps.append({"xh": xh_l, "xl": xl8_l, "x8": x8_l, "w16": w16_l, "wl": wl8_l, "btab": btab})
    return in_maps


def kernel(hidden_states, weight, expert_bias, _trace=False):
    from concourse.bass_utils import run_bass_kernel_spmd

    if "nc" not in _cache:
        _cache["nc"] = _build()
    nc = _cache["nc"]
    in_maps = _prep(hidden_states, weight, expert_bias)
    res = run_bass_kernel_spmd(nc, in_maps, core_ids=list(range(NCORES)), trace=_trace)
    _cache["last_results"] = res
    w = np.concatenate([res.results[c]["w_out"] for c in range(NCORES)], axis=0)
    idx = np.concatenate([res.results[c]["i_out"] for c in range(NCORES)], axis=0)
    return w.astype(np.float32), idx.astype(np.int32)


# revision 28
# speedup vs baseline: 1.0646x; 1.0306x over previous
"""MoE gate (LLaDA2) routing kernel for 8 Trainium2 NeuronCores.

Token-parallel over 8 cores (2048 tokens/core, 16 tiles of 128). Router GEMM
runs as an fp16 main pass (xh @ w16*2^19) plus a DoubleRow fp8 correction
(x8@wl8*2^19 + xl8*2^12 @ w8*2^7), all accumulating into ONE PSUM bank per
tile at 2^19 scale, so the sigmoid reads PSUM directly with scale=2^-19 and
no combine/copy ops are needed. w8 (= fp8(w*2^7)) is derived on-chip once.
PE schedule: warm-up junk matmuls (HAM clock-gate release), then per-tile
[main, correction] pairs with corrections starting as early as DMA allows,
giving the DVE-bound epilogues the widest window. Grouped top-8 routing
epilogue: all 256-wide work on DVE (GPSIMD is ~8ns/elem/partition -- only
<=64-elem ops go to Pool; its custom-lib ops cost a ~7us library swap, so
none are used), ACT does sigmoid + final scale-by-reciprocal copies; the
normalization reciprocals run batched at the end (dens split 12/4 so the
first 12 tiles' outputs stream out early), and the last tile's reorder
trees run on DVE to shorten the kernel tail.
"""
import sys
for p in ("/opt/trn_rl_repo", "/root/.axon_site/_ro/trn_rl_repo"):
    if p not in sys.path:
        sys.path.append(p)

import numpy as np

T, H, E = 16384, 4096, 256
NCORES = 8
TPC = T // NCORES          # tokens per core: 2048
NTILES = TPC // 128        # 16 row tiles
KCH = H // 128             # 32 contraction chunks
KQ = KCH // 4              # chunks per w16 quarter
G = 8                      # expert groups
GS = E // G                # 32 experts/group
K = 8                      # top-k
BIG = 2.0 ** 100
NEG = -1.0e4
SCALE = 2.0 ** -19         # undo the 2^19 logit scaling at sigmoid time
NWARM = 30                 # junk matmuls to release the PE HAM clock gate
AHEAD = 8                  # main passes run before the first correction pass

_cache = {}


def _build():
    import concourse.bacc as bacc
    import concourse.bass as bass
    import concourse.mybir as mybir
    from concourse import tile

    dt = mybir.dt
    Alu = mybir.AluOpType
    Act = mybir.ActivationFunctionType
    Ax = mybir.AxisListType
    DR = mybir.MatmulPerfMode.DoubleRow

    nc = bacc.Bacc("TRN2", target_bir_lowering=False, debug=False,
                   num_devices=NCORES)

    xh_d = nc.dram_tensor("xh", [NTILES, 128, KCH, 128], dt.float16, kind="ExternalInput")
    xl_d = nc.dram_tensor("xl", [NTILES, 128, KCH, 128], dt.float8e4, kind="ExternalInput")
    x8_d = nc.dram_tensor("x8", [NTILES, 128, KCH, 128], dt.float8e4, kind="ExternalInput")
    w16_d = nc.dram_tensor("w16", [128, KCH, E], dt.float16, kind="ExternalInput")
    wl_d = nc.dram_tensor("wl", [128, KCH, E], dt.float8e4, kind="ExternalInput")
    btab_d = nc.dram_tensor("btab", [128, E], dt.float32, kind="ExternalInput")
    w_out = nc.dram_tensor("w_out", [TPC, K], dt.float32, kind="ExternalOutput")
    i_out = nc.dram_tensor("i_out", [TPC, K], dt.uint32, kind="ExternalOutput")

    def bc_mid(ap8, n=8):
        # [128, m] -> [128, n(bcast), m]
        return bass.AP(ap8.tensor, ap8.offset, [list(ap8.ap[0]), [0, n], list(ap8.ap[1])])

    with tile.TileContext(nc) as tc:
        with (
            tc.tile_pool(name="wpool", bufs=1) as wpool,
            tc.tile_pool(name="xpool", bufs=6) as xpool,
            tc.tile_pool(name="lpool", bufs=12) as lpool,
            tc.tile_pool(name="qpool", bufs=9) as qpool,
            tc.tile_pool(name="ppool", bufs=8, space="PSUM") as ppool,
            tc.tile_pool(name="spool", bufs=5) as spool,
            tc.tile_pool(name="tpool", bufs=5) as tpool,
            tc.tile_pool(name="opool", bufs=1) as opool,
            tc.tile_pool(name="dpool", bufs=16) as dpool,
        ):
            # ---------------- input DMAs, in delivery-priority order -------
            w16q = [wpool.tile([128, KQ * E], dt.float16, tag=f"w16q{q}", name=f"w16q{q}") for q in range(4)]
            w16_flat = w16_d[:].rearrange("p k e -> p (k e)")
            xts, xlts = [None] * NTILES, [None] * NTILES
            x8ts = [None] * NTILES

            def dma_x(i):
                xt = xpool.tile([128, KCH * 128], dt.float16, tag="x")
                nc.sync.dma_start(xt[:], xh_d[i].rearrange("p k t -> p (k t)"))
                xts[i] = xt

            def dma_xl(i):
                lt = lpool.tile([128, KCH * 128], dt.float8e4, tag="xl")
                nc.sync.dma_start(lt[:], xl_d[i].rearrange("p k t -> p (k t)"))
                xlts[i] = lt
                qt = qpool.tile([128, KCH * 128], dt.float8e4, tag="x8")
                nc.sync.dma_start(qt[:], x8_d[i].rearrange("p k t -> p (k t)"))
                x8ts[i] = qt

            nc.sync.dma_start(w16q[0][:], w16_flat[:, 0:KQ * E])
            dma_x(0)
            nc.sync.dma_start(w16q[1][:], w16_flat[:, KQ * E:2 * KQ * E])
            nc.sync.dma_start(w16q[2][:], w16_flat[:, 2 * KQ * E:3 * KQ * E])
            nc.sync.dma_start(w16q[3][:], w16_flat[:, 3 * KQ * E:4 * KQ * E])
            wl_flat = wl_d[:].rearrange("p k e -> p (k e)")
            dma_xl(0)
            wl8a = wpool.tile([128, KCH * E // 2], dt.float8e4, tag="wl8a")
            nc.sync.dma_start(wl8a[:], wl_flat[:, 0:KCH * E // 2])
            wl8b = wpool.tile([128, KCH * E // 2], dt.float8e4, tag="wl8b")
            nc.sync.dma_start(wl8b[:], wl_flat[:, KCH * E // 2:])
            btab = wpool.tile([128, E], dt.float32, tag="btab")
            nc.sync.dma_start(btab[:], btab_d[:])
            for i in range(1, NTILES):
                dma_x(i)
                dma_xl(i)

            # ---------------- PE warm-up (no data deps) --------------------
            junk = wpool.tile([128, 384], dt.float16, tag="junk")
            nc.vector.memset(junk[:], 0.0)
            jp = ppool.tile([128, E], dt.float32, tag="ps")
            for _ in range(NWARM):
                nc.tensor.matmul(jp[:], lhsT=junk[:, 0:128], rhs=junk[:, 128:384],
                                 start=True, stop=True)

            # ---------------- on-chip fp8 derivations (DVE) ----------------
            # w8 = fp8(w * 2^7) = fp8(w16 * 2^-12), one-time
            w8d = wpool.tile([128, KCH * E], dt.float8e4, tag="w8d")
            for q in range(4):
                nc.vector.tensor_scalar(w8d[:, q * KQ * E:(q + 1) * KQ * E],
                                        w16q[q][:], 2.0 ** -12, None, op0=Alu.mult)

            psums = [None] * NTILES
            w8s = [None] * NTILES
            out_w = opool.tile([128, NTILES * K], dt.float32, tag="ow")
            dtileA = opool.tile([128, 12], dt.float32, tag="densA")
            dtileB = opool.tile([128, 4], dt.float32, tag="densB")
            rtile = opool.tile([128, NTILES], dt.float32, tag="recs")
            out_i = opool.tile([128, NTILES * K], dt.uint32, tag="oi")

            def main_half(i, h):
                if h == 0:
                    psums[i] = ppool.tile([128, E], dt.float32, tag="ps", name="ps")
                ps = psums[i]
                x = xts[i]
                for k in range(h * KCH // 2, (h + 1) * KCH // 2):
                    nc.tensor.matmul(ps[:],
                                     lhsT=x[:, k * 128:(k + 1) * 128],
                                     rhs=w16q[k // KQ][:, (k % KQ) * E:(k % KQ + 1) * E],
                                     start=(k == 0), stop=False)

            def main_pass(i):
                main_half(i, 0)
                main_half(i, 1)

            def corr_pass_a(i):
                # xl8 x w8d half first: its operands (xl8_i + derived w8d)
                # land before wl8, so the first correction starts sooner
                ps = psums[i]
                l3 = xlts[i][:].rearrange("p (k t) -> p k t", k=KCH)
                vb = w8d[:].rearrange("p (k e) -> p k e", k=KCH)
                for j in range(KCH // 2):
                    nc.tensor.matmul(ps[:], lhsT=l3[:, 2 * j:2 * j + 2, :],
                                     rhs=vb[:, 2 * j:2 * j + 2, :],
                                     start=False, stop=False, perf_mode=DR)

            def corr_pass_b(i):
                ps = psums[i]
                u3 = x8ts[i][:].rearrange("p (k t) -> p k t", k=KCH)
                va = wl8a[:].rearrange("p (k e) -> p k e", k=KCH // 2)
                vc = wl8b[:].rearrange("p (k e) -> p k e", k=KCH // 2)
                for j in range(KCH // 4):
                    nc.tensor.matmul(ps[:], lhsT=u3[:, 2 * j:2 * j + 2, :],
                                     rhs=va[:, 2 * j:2 * j + 2, :],
                                     start=False, stop=False, perf_mode=DR)
                for j in range(KCH // 4):
                    nc.tensor.matmul(ps[:], lhsT=u3[:, 16 + 2 * j:16 + 2 * j + 2, :],
                                     rhs=vc[:, 2 * j:2 * j + 2, :],
                                     start=False, stop=(j == KCH // 4 - 1), perf_mode=DR)

            def corr_pass(i):
                corr_pass_a(i)
                corr_pass_b(i)

            def epilogue(i):
                ps = psums[i]
                scores = spool.tile([128, E], dt.float32, tag="scores")
                nc.scalar.activation(scores[:], ps[:], Act.Sigmoid, scale=SCALE)
                sr = spool.tile([128, E], dt.float32, tag="sr")
                nc.vector.tensor_tensor(sr[:], scores[:], btab[:], Alu.add)
                sr3 = sr[:].rearrange("p (g e) -> p g e", g=G)

                top1 = tpool.tile([128, G], dt.float32, tag="top1")
                nc.vector.tensor_reduce(top1[:], sr3, axis=Ax.X, op=Alu.max)
                mr2 = spool.tile([128, E], dt.float32, tag="mr2")
                nc.vector.match_replace(mr2[:], in_to_replace=top1[:], in_values=sr[:], imm_value=NEG)
                top2 = tpool.tile([128, G], dt.float32, tag="top2")
                nc.vector.tensor_reduce(top2[:], mr2[:].rearrange("p (g e) -> p g e", g=G), axis=Ax.X, op=Alu.max)
                gs_t = tpool.tile([128, G], dt.float32, tag="gs")
                nc.vector.tensor_tensor(gs_t[:], top1[:], top2[:], Alu.add)
                g8 = tpool.tile([128, 8], dt.float32, tag="g8")
                nc.vector.max(out=g8[:], in_=gs_t[:])
                # inv = 1e4 for groups strictly below the 4th-largest group score
                inv = tpool.tile([128, G], dt.float32, tag="inv")
                nc.vector.tensor_scalar(inv[:], gs_t[:], g8[:, 3:4], -NEG, op0=Alu.is_lt, op1=Alu.mult)
                nc.vector.tensor_tensor(sr3, sr3, inv[:].to_broadcast([128, G, GS]), Alu.subtract)

                vals8 = tpool.tile([128, K], dt.float32, tag="vals8")
                nc.vector.max(out=vals8[:], in_=sr[:])
                oi_c = out_i[:, i * K:(i + 1) * K]
                nc.vector.max_index(out=oi_c, in_max=vals8[:], in_values=sr[:])

                # sel = scores at the selected positions (threshold at the 8th value)
                sel = spool.tile([128, E], dt.float32, tag="sel")
                nc.vector.scalar_tensor_tensor(sel[:], in0=sr[:], scalar=vals8[:, 7:8],
                                               in1=scores[:], op0=Alu.is_ge, op1=Alu.mult)
                svals8 = tpool.tile([128, K], dt.float32, tag="svals8")
                nc.vector.max(out=svals8[:], in_=sel[:])
                sidx8 = tpool.tile([128, K], dt.uint32, tag="sidx8")
                nc.vector.max_index(out=sidx8[:], in_max=svals8[:], in_values=sel[:])

                # reorder svals8 (score order) into routing order: K x K match
                eq = tpool.tile([128, K * K], dt.float32, tag="eq")
                eq3 = eq[:].rearrange("p (k j) -> p k j", k=K)
                nc.vector.tensor_tensor(eq3, oi_c.to_broadcast([128, K, K]), bc_mid(sidx8[:]), Alu.is_equal)
                # the K x K trees run on the idle Pool engine, except the last
                # tile where the faster DVE shortens the kernel tail
                eng = nc.vector if i == NTILES - 1 else nc.gpsimd
                eng.tensor_tensor(eq3, eq3, bc_mid(svals8[:]), Alu.mult)
                w84 = tpool.tile([128, K * 4], dt.float32, tag="w84")
                w843 = w84[:].rearrange("p (k j) -> p k j", k=K)
                eng.tensor_tensor(w843, eq3[:, :, 0:4], eq3[:, :, 4:8], Alu.add)
                w82 = tpool.tile([128, K * 2], dt.float32, tag="w82")
                w823 = w82[:].rearrange("p (k j) -> p k j", k=K)
                eng.tensor_tensor(w823, w843[:, :, 0:2], w843[:, :, 2:4], Alu.add)
                w8 = dpool.tile([128, K], dt.float32, tag="w8")
                eng.tensor_tensor(w8[:].rearrange("p (k o) -> p k o", o=1),
                                  w823[:, :, 0:1], w823[:, :, 1:2], Alu.add)

                s4 = tpool.tile([128, 4], dt.float32, tag="s4")
                eng.tensor_tensor(s4[:], w8[:, 0:4], w8[:, 4:8], Alu.add)
                s2 = tpool.tile([128, 2], dt.float32, tag="s2")
                eng.tensor_tensor(s2[:], s4[:, 0:2], s4[:, 2:4], Alu.add)
                dc = dtileA[:, i:i + 1] if i < 12 else dtileB[:, i - 12:i - 11]
                eng.tensor_tensor(dc, s2[:, 0:1], s2[:, 1:2], Alu.add)
                eng.tensor_scalar(dc, dc, 1.0 / 2.5, None, op0=Alu.mult)
                w8s[i] = w8

            # ---------------- schedule ------------------------------------
            # Per-tile [main, correction] pairs, corrections starting as early
            # as DMA allows so the (DVE-bound) epilogues get the widest
            # possible window. All normalize/scale finishes run at the very
            # end so the DVE stream never blocks mid-run on the Pool trees.
            main_pass(0)
            corr_pass_a(0)
            main_half(1, 0)
            main_half(1, 1)
            corr_pass_b(0)
            epilogue(0)
            corr_pass(1)
            epilogue(1)
            for i in range(2, NTILES):
                main_pass(i)
                corr_pass(i)
                epilogue(i)
            nc.vector.reciprocal(rtile[:, 0:12], dtileA[:])
            for t in range(12):
                nc.scalar.activation(out_w[:, t * K:(t + 1) * K], w8s[t][:], Act.Copy,
                                     scale=rtile[:, t:t + 1])
            wo = w_out[:].rearrange("(i p) k -> p i k", p=128)[:, 0:12, :]
            io = i_out[:].rearrange("(i p) k -> p i k", p=128)[:, 0:12, :]
            nc.sync.dma_start(wo, out_w[:, 0:96].rearrange("p (i k) -> p i k", i=12))
            nc.sync.dma_start(io, out_i[:, 0:96].rearrange("p (i k) -> p i k", i=12))
            nc.vector.reciprocal(rtile[:, 12:16], dtileB[:])
            for t in range(12, NTILES):
                nc.scalar.activation(out_w[:, t * K:(t + 1) * K], w8s[t][:], Act.Copy,
                                     scale=rtile[:, t:t + 1])
            wo = w_out[:].rearrange("(i p) k -> p i k", p=128)[:, 12:16, :]
            io = i_out[:].rearrange("(i p) k -> p i k", p=128)[:, 12:16, :]
            nc.sync.dma_start(wo, out_w[:, 96:128].rearrange("p (i k) -> p i k", i=4))
            nc.sync.dma_start(io, out_i[:, 96:128].rearrange("p (i k) -> p i k", i=4))

    nc.compile()
    return nc


def _prep(hidden_states, weight, expert_bias):
    import ml_dtypes
    f8 = ml_dtypes.float8_e4m3
    x = np.ascontiguousarray(hidden_states, dtype=np.float32)
    w = np.ascontiguousarray(weight, dtype=np.float32)

    w16s = (w * 2.0 ** 19).astype(np.float16)            # shipped main weights
    wl = w - w16s.astype(np.float32) * 2.0 ** -19
    wl8 = (wl * 2.0 ** 19).astype(f8)                    # scaled residual of w
    w16_l = np.ascontiguousarray(w16s.reshape(E, KCH, 128).transpose(2, 1, 0))
    wl8_l = np.ascontiguousarray(wl8.reshape(E, KCH, 128).transpose(2, 1, 0))
    btab = np.ascontiguousarray(np.broadcast_to(expert_bias.astype(np.float32), (128, E)))

    in_maps = []
    for c in range(NCORES):
        xs = x[c * TPC:(c + 1) * TPC]
        xh16 = xs.astype(np.float16)
        xl = xs - xh16.astype(np.float32)
        xl8 = (xl * 2.0 ** 12).astype(f8)
        x8 = xs.astype(f8)
        xh_l = np.ascontiguousarray(xh16.reshape(NTILES, 128, KCH, 128).transpose(0, 3, 2, 1))
        xl8_l = np.ascontiguousarray(xl8.reshape(NTILES, 128, KCH, 128).transpose(0, 3, 2, 1))
        x8_l = np.ascontiguousarray(x8.reshape(NTILES, 128, KCH, 128).transpose(0, 3, 2, 1))
        in_maps.append({"xh": xh_l, "xl": xl8_l, "x8": x8_l, "w16": w16_l, "wl": wl8_l, "btab": btab})
    return in_maps


def kernel(hidden_states, weight, expert_bias, _trace=False):
    from concourse.bass_utils import run_bass_kernel_spmd

    if "nc" not in _cache:
        _cache["nc"] = _build()
    nc = _cache["nc"]
    in_maps = _prep(hidden_states, weight, expert_bias)
    res = run_bass_kernel_spmd(nc, in_maps, core_ids=list(range(NCORES)), trace=_trace)
    _cache["last_results"] = res
    w = np.concatenate([res.results[c]["w_out"] for c in range(NCORES)], axis=0)
    idx = np.concatenate([res.results[c]["i_out"] for c in range(NCORES)], axis=0)
    return w.astype(np.float32), idx.astype(np.int32)
